# revision 1
# baseline (speedup 1.0000x reference)
"""AttentionMemory kernel for Trainium2 (8 NeuronCores, Bass/Tile).

Reference computation (per batch b):
    affinity[n, m] = (2 * mk[:,n]@qk[:,m] - ||mk[:,n]||^2 - ||qk[:,m]||^2) / 8
    out[n, m]      = softmax over n (memory axis)

Softmax over n is invariant to per-column constants, so the -||qk_m||^2
term is dropped.  Logits are produced by an augmented float32r matmul:
    lhsT (stationary) = [0.25 * qk ; -0.125 ; -0.125]   -> [66, Mc]
    rhs  (moving)     = [mk        ; a1     ; a2     ]  -> [66, N]
    psum[m, n] = 0.25*dot(qk_m, mk_n) - 0.125*(a1+a2)_n == logits[m, n]
with a = sum_c mk[c,n]^2 split on the host into a1 (10-mantissa-bit
exact, safe under any f32r rounding) + a2 (residual).

float32r runs at 1 cycle/row for moving free dim >= 256 (vs 3 bf16
hi/lo matmuls previously), with >= 10 mantissa bits; end-to-end metric
error is ~2e-3, dominated by the bf16 output store.

Sharding: core c handles batch c//2, query-column half c%2 (communication
free: softmax is over the full n axis which each core holds).  Each core
writes out_c[m, n] in bf16; the host upcasts and transposes to the
reference [n, m] f32 layout.

Pipeline per 126-row query strip: PE (4 f32r matmuls per 2016-col piece)
-> ACT (exp PSUM->SBUF bf16, the critical path) -> SP HWDGE store of the
UNNORMALIZED exp per piece.  The softmax denominator is recovered on the
host (Z = sum of the stored bf16 exps), so the device runs no normalize
pass, stores never wait on the row-sum, and the ACT stream carries no
accumulator-read auxes.

For DVE_STRIPS, the trailing 1008-col piece bypasses ACT entirely: the DVE computes
e^x = 2^i * q(r) with i = rint(x*log2e) (int16 convert), 2^i built as a
bf16 bit pattern (i*128 + 16256, bitcast), r = x - i*ln2, and q a minimax
quadratic -- ~5e-3 max rel error.  The chain stays on the DVE (the tile
tracker misses tensor_tensor-class writes; same-engine program order is
the correctness guarantee) and ends in a tracked tensor_scalar copy; its
store rides the gpsimd SWDGE ring so the in-order SP store queue is not
head-of-line blocked.
"""

import math

import numpy as np

B, CK, H, W = 4, 64, 48, 84
N = H * W            # 4032 memory pixels (softmax axis)
HALF = N // 2        # 2016 query pixels per core
M_STRIP = 126        # output-partition strip size (16 * 126 = 2016)
N_STRIPS = HALF // M_STRIP
K_AUG = CK + 2       # 66: contraction dim incl. the two -a rows

PIECE = 2016         # ACT exp granularity: 4 PSUM banks (4 x 504 chunks)
N_CHUNK = 504        # matmul moving free dim (one PSUM bank, 8 pad cols)
N_CHUNKS = N // N_CHUNK  # 8

_CACHE = {}
STRIP0_PIECES = [1, 1, 1, 2, 1, 2]
STRIPL_PIECES = [3, 3, 2]
STRIP1_PIECES = [4, 4]
# Strips use pieces [3,3,2] (chunks): two 3-bank main pieces on the "ps"
# ring plus a trailing 2-chunk piece in its own dedicated 2-bank PSUM slot
# ("ps2", 2*3 + 2 = 8 banks).  ACT always drains exactly two main pieces
# per strip, so handing a strip's ps2 piece to the DVE exp chain does not
# shift ACT's phase against the PE psum refill (no stream stall); each
# DVE strip sheds a full 1025 ns of ACT.  DVE_STRIPS alternates strips so
# the ~5.3 us chains never backlog.  exp(r) quadratic minimax coefficients
# on [-ln2/2, ln2/2].
DVE_STRIPS = (1, 2, 4, 6, 8, 9, 11, 13)
DVE_COLS = (0,)
EXP_BUFS = 3
M_ODD_ON_POOL = False
E16_ON_POOL = True
HH_ON_POOL = False
DV_ON_POOL = False
N_WARMUP = 6
M_CHUNK_PLAN = [(0, 1), (1, 3), (3, 5), (5, 8)]
MERGE_MAIN_STORES = False
Q_REST_POS = 99
DVE_BUFS = 2
Q_ON_POOL = True
LAST_ON_POOL = False
LAST_ON_ACT = False
_C0, _C1, _C2 = 1.0004425609008205, 1.0148395834554758, 0.49624184716972364
LOG2E = 1.0 / math.log(2.0)
LN2 = math.log(2.0)


def _build_nc():
    import concourse.bacc as bacc
    import concourse.mybir as mybir
    import concourse.tile as tile

    f32 = mybir.dt.float32
    f32r = mybir.dt.float32r
    bf16 = mybir.dt.bfloat16
    i16 = mybir.dt.int16
    Exp = mybir.ActivationFunctionType.Exp
    Alu = mybir.AluOpType

    nc = bacc.Bacc("TRN2", target_bir_lowering=False, debug=False)

    q_d = nc.dram_tensor("q", [K_AUG, HALF], f32r, kind="ExternalInput")
    m_d = nc.dram_tensor("m", [K_AUG, N], f32r, kind="ExternalInput")
    out_d = nc.dram_tensor("out_c", [HALF, N], bf16, kind="ExternalOutput")

    with tile.TileContext(nc) as tc:
        with (
            tc.tile_pool(name="singles", bufs=1) as singles,
            tc.tile_pool(name="psum", bufs=2, space="PSUM") as psum_pool,
            tc.tile_pool(name="psum2", bufs=1, space="PSUM") as psum2_pool,
            tc.tile_pool(name="exp", bufs=EXP_BUFS) as exp_pool,
            tc.tile_pool(name="dve_out", bufs=DVE_BUFS) as dve_pool,
        ):
            # --- inputs, staged by first use so the pipeline head starts as
            # early as possible.  The first q strip-pair rides the ACT HWDGE
            # ring so its dispatch overlaps the SP ring; m arrives in 504-col
            # chunks so the first matmul waits on 1/8 of it ------------------
            q_s = singles.tile([K_AUG, HALF], f32r)
            m_s = singles.tile([K_AUG, N], f32r)
            if Q_ON_POOL:
                nc.gpsimd.dma_start(out=q_s[:, :252], in_=q_d[:, :252])
            else:
                nc.scalar.dma_start(out=q_s[:, :252], in_=q_d[:, :252])
            # early chunks arrive singly (pipeline head wants them ASAP);
            # late chunks merge into bigger DMAs so fewer HWDGE descriptor
            # generations (625 ns each, serialized device) block the first
            # stores' DGE behind the input cascade
            for i, (c0c, c1c) in enumerate(M_CHUNK_PLAN):
                sl = slice(c0c * N_CHUNK, c1c * N_CHUNK)
                nc.sync.dma_start(out=m_s[:, sl], in_=m_d[:, sl])
                if i == Q_REST_POS:
                    nc.sync.dma_start(out=q_s[:, 252:], in_=q_d[:, 252:])
            if Q_REST_POS >= len(M_CHUNK_PLAN):
                nc.sync.dma_start(out=q_s[:, 252:], in_=q_d[:, 252:])

            # --- prewarm: ACT exp table load + PE pstate ramp during the
            # input DMAs -----------------------------------------------------
            wtab = singles.tile([1, 2], f32)
            nc.vector.memset(wtab, 0.0)
            nc.scalar.activation(wtab[:, 1:2], wtab[:, 0:1], Exp)
            wsrc = singles.tile([K_AUG, 256], bf16)
            nc.vector.memset(wsrc, 0.0)
            # scratch for the DVE exp chain (reused serially; DVE program
            # order guarantees correctness, final tensor_scalar write into
            # exp_t carries the tracked dependency for the store)
            xc = singles.tile([M_STRIP, PIECE], f32)
            t16 = singles.tile([M_STRIP, PIECE], i16)
            e16 = singles.tile([M_STRIP, PIECE], i16)
            rr = singles.tile([M_STRIP, PIECE], bf16)
            hh = singles.tile([M_STRIP, PIECE], bf16)
            gg = singles.tile([M_STRIP, PIECE], bf16)
            yy = singles.tile([M_STRIP, PIECE], bf16)

            wps = psum_pool.tile([M_STRIP, 1536], f32, tag="ps")
            for _ in range(N_WARMUP):
                nc.tensor.matmul(
                    wps[:, :256],
                    wsrc[:, :M_STRIP],
                    wsrc,
                    start=True,
                    stop=True,
                )

            E16_ENG = nc.gpsimd.tensor_scalar if E16_ON_POOL else nc.vector.tensor_scalar
            HH_ENG = nc.gpsimd.tensor_scalar if HH_ON_POOL else nc.vector.tensor_scalar
            DV_ENG = nc.gpsimd.tensor_scalar if DV_ON_POOL else nc.vector.tensor_scalar
            for s in range(N_STRIPS):
                m0 = s * M_STRIP
                q_l = q_s[:, m0 : m0 + M_STRIP]

                exp_t = exp_pool.tile([M_STRIP, N], bf16, tag="exp")

                # ACT pieces in 504-col chunk counts.  Strip 0 ramps up so
                # the first exp starts right after the first matmul; the last
                # strip ramps down so the final store transfer is short.
                if s == 0:
                    pieces = STRIP0_PIECES
                elif s == N_STRIPS - 1:
                    pieces = STRIPL_PIECES
                else:
                    pieces = [3, 3, 2]

                c0 = 0
                for k in pieces:
                    # one PSUM bank (512 cols) per 504-wide chunk; each chunk
                    # starts on a bank boundary — PE writes must not straddle
                    # a bank.  The trailing 2-chunk piece lives in its own
                    # 2-bank slot so ACT's main-slot cadence is independent
                    # of who drains it (no stream-phase stall on DVE strips)
                    if k == 2 and c0 == 6:
                        ps = psum2_pool.tile([M_STRIP, 1024], f32, tag="ps2")
                    else:
                        ps = psum_pool.tile([M_STRIP, 512 * k], f32, tag="ps")
                    for j in range(k):
                        sl = slice((c0 + j) * N_CHUNK, (c0 + j + 1) * N_CHUNK)
                        nc.tensor.matmul(
                            ps[:, j * 512 : j * 512 + N_CHUNK],
                            q_l,
                            m_s[:, sl],
                            start=True,
                            stop=True,
                        )
                    # exp(logits) PSUM->SBUF bf16; the strided 3D views skip
                    # the 8 pad cols per bank
                    e0 = c0 * N_CHUNK
                    sl = slice(e0, e0 + k * N_CHUNK)
                    if s in DVE_STRIPS and c0 == 6:
                        # DVE path: e^x = 2^i * q(r), i = rint(x*log2e),
                        # r = x - i*ln2; 2^i built directly as a bf16 bit
                        # pattern, q the minimax quadratic
                        w = k * N_CHUNK
                        psv = ps.rearrange("p (b c) -> p b c", b=k)[:, :, :N_CHUNK]
                        nc.vector.tensor_scalar(
                            t16[:, :w].rearrange("p (b c) -> p b c", b=k), psv,
                            LOG2E, None, Alu.mult,
                        )
                        E16_ENG(e16[:, :w], t16[:, :w], 128, 16256, Alu.mult, Alu.add)
                        nc.vector.scalar_tensor_tensor(
                            rr[:, :w].rearrange("p (b c) -> p b c", b=k),
                            t16[:, :w].rearrange("p (b c) -> p b c", b=k),
                            -LN2, psv, Alu.mult, Alu.add,
                        )
                        HH_ENG(hh[:, :w], rr[:, :w], _C2, _C1, Alu.mult, Alu.add)
                        nc.vector.scalar_tensor_tensor(gg[:, :w], hh[:, :w], 1.0, rr[:, :w], Alu.mult, Alu.mult)
                        nc.vector.scalar_tensor_tensor(
                            yy[:, :w], gg[:, :w], _C0, e16[:, :w].bitcast(bf16), Alu.add, Alu.mult
                        )
                        # dedicated output tile so the late DVE write does
                        # not hold the strip's exp_t buffer rotation
                        dv = dve_pool.tile([M_STRIP, w], bf16, tag="dv")
                        DV_ENG(dv, yy[:, :w], 1.0, None, Alu.mult)
                        # gpsimd SWDGE ring: keeps this late store out of the
                        # in-order SP store queue (Pool is otherwise idle)
                        nc.gpsimd.dma_start(
                            out=out_d[m0 : m0 + M_STRIP, sl], in_=dv
                        )
                    else:
                        nc.scalar.activation(
                            exp_t[:, sl].rearrange("p (b c) -> p b c", b=k),
                            ps.rearrange("p (b c) -> p b c", b=k)[:, :, :N_CHUNK],
                            Exp,
                        )
                        if LAST_ON_ACT and s == N_STRIPS - 1 and c0 >= 4:
                            nc.scalar.dma_start(
                                out=out_d[m0 : m0 + M_STRIP, sl],
                                in_=exp_t[:, sl],
                            )
                        elif LAST_ON_POOL and s == N_STRIPS - 1 and c0 >= 4:
                            nc.gpsimd.dma_start(
                                out=out_d[m0 : m0 + M_STRIP, sl],
                                in_=exp_t[:, sl],
                            )
                        else:
                            st = sl
                            if MERGE_MAIN_STORES and pieces == [3, 3, 2]:
                                if c0 == 0:
                                    st = None  # deferred into the c0==3 store
                                elif c0 == 3:
                                    st = slice(0, 6 * N_CHUNK)
                            if st is not None:
                                nc.sync.dma_start(
                                    out=out_d[m0 : m0 + M_STRIP, st],
                                    in_=exp_t[:, st],
                                )
                    c0 += k

    nc.compile()
    return nc


def _get_nc():
    if "nc" not in _CACHE:
        _CACHE["nc"] = _build_nc()
    return _CACHE["nc"]


def _round_mant(x: np.ndarray, bits: int) -> np.ndarray:
    """Round to `bits` explicit mantissa bits (exact under f32r rounding)."""
    m, e = np.frexp(x.astype(np.float64))
    scale = 2.0 ** (bits + 1)
    return np.ldexp(np.round(m * scale) / scale, e).astype(np.float32)


def kernel(mk: np.ndarray, qk: np.ndarray) -> np.ndarray:
    from concourse import bass_utils

    mk = np.asarray(mk, dtype=np.float32).reshape(B, CK, N)
    qk = np.asarray(qk, dtype=np.float32).reshape(B, CK, N)
    a = np.einsum("bcn,bcn->bn", mk.astype(np.float64), mk.astype(np.float64))
    a1 = _round_mant(a, 10)
    a2 = (a - a1).astype(np.float32)

    in_maps = []
    for core in range(8):
        b, h = divmod(core, 2)
        m_aug = np.empty((K_AUG, N), np.float32)
        m_aug[:CK] = mk[b]
        m_aug[CK] = a1[b]
        m_aug[CK + 1] = a2[b]

        q_aug = np.empty((K_AUG, HALF), np.float32)
        q_aug[:CK] = 0.25 * qk[b, :, h * HALF : (h + 1) * HALF]
        q_aug[CK:] = -0.125

        in_maps.append({"q": q_aug, "m": m_aug})

    res = bass_utils.run_bass_kernel_spmd(
        _get_nc(), in_maps, core_ids=list(range(8))
    )
    _CACHE["last_results"] = res

    out = np.empty((B, N, N), np.float32)
    for core in range(8):
        b, h = divmod(core, 2)
        e = res.results[core]["out_c"].astype(np.float32)  # [m, n] unnormalized
        e /= e.sum(axis=1, keepdims=True)
        out[b, :, h * HALF : (h + 1) * HALF] = e.T
    return out

